# revision 28
# baseline (speedup 1.0000x reference)
"""DIN (attention pooling + MLP w/ BatchNorm+Dice) Trainium2 kernel, 8-core SPMD.

Contract: kernel(**inputs) takes FULL unsharded inputs (numpy), returns [4096,1] f32.
Internally: batch-sharded 512 samples/core; embedding tables + weights replicated.

Tables are uploaded bf16; history embeddings for a 128-sample tile are fetched
with a single batched indirect DMA (offset AP [128,100]); attention runs on DVE
in bf16 (product w/ broadcast, in-place log-tree over d, per-h tensor_scalar
scale, log-tree over h).
"""
import sys
sys.path.insert(0, "/opt/trn_rl_repo")
import numpy as np
import ml_dtypes

import concourse.bass as bass
import concourse.mybir as mybir
import concourse.tile as tile
from concourse import bacc
from concourse.bass_utils import run_bass_kernel_spmd
from concourse.masks import make_identity

B, H, D = 4096, 100, 128
N_ITEM, M_USER = 100000, 500000
D1, D2 = 1024, 512
NCORES = 8
BL = B // NCORES          # 512 samples per core
T = BL // 128             # 4 sample-tiles per core
DICE_EPS, BN_EPS = 1e-3, 1e-5

BF = mybir.dt.bfloat16
F32 = mybir.dt.float32
I32 = mybir.dt.int32
MUL = mybir.AluOpType.mult
ADD = mybir.AluOpType.add
SUB = mybir.AluOpType.subtract
AF = mybir.ActivationFunctionType

_PROG = None


def _r3(t_ap):
    return t_ap.rearrange("p (h d) -> p h d", d=D)


def _bcol(col_ap, n):
    """[128,1] AP -> [128,n] broadcast AP (step-0 inner dim)."""
    return col_ap.to_broadcast([col_ap.shape[0], n])


def _build(sim_mode=False):
    ndev = 1 if sim_mode else NCORES
    nc = bacc.Bacc("TRN2", target_bir_lowering=False, debug=False, num_devices=ndev)

    # Tables are declared with a wide inner dim (same row-major bytes); gather
    # indices are pre-scaled by D on the host and used with axis=1 (coef=1),
    # so each index is a flat element offset into the table.
    user_tab = nc.dram_tensor("user_table", [M_USER * D // 4096, 4096], BF,
                              kind="ExternalInput")
    item_tab = nc.dram_tensor("item_table", [N_ITEM * D // 4096, 4096], BF,
                              kind="ExternalInput")
    idx_hist = nc.dram_tensor("idx_hist", [128, T * H], I32, kind="ExternalInput")
    idx_u = nc.dram_tensor("idx_u", [128, T], I32, kind="ExternalInput")
    idx_i = nc.dram_tensor("idx_i", [128, T], I32, kind="ExternalInput")
    w1d = nc.dram_tensor("w1sb", [128, 3 * D1], BF, kind="ExternalInput")
    w2d = nc.dram_tensor("w2sb", [128, 8 * D2], BF, kind="ExternalInput")
    w3d = nc.dram_tensor("w3sb", [128, 4], BF, kind="ExternalInput")
    g1d = nc.dram_tensor("g1r", [128, 8], F32, kind="ExternalInput")
    be1d = nc.dram_tensor("be1r", [128, 8], F32, kind="ExternalInput")
    g2d = nc.dram_tensor("g2r", [128, 4], F32, kind="ExternalInput")
    be2d = nc.dram_tensor("be2r", [128, 4], F32, kind="ExternalInput")
    a1d = nc.dram_tensor("a1c", [128, 1], F32, kind="ExternalInput")
    a2d = nc.dram_tensor("a2c", [128, 1], F32, kind="ExternalInput")
    b3d = nc.dram_tensor("b3c", [1, 1], F32, kind="ExternalInput")
    outd = nc.dram_tensor("out", [1, BL], F32, kind="ExternalOutput")

    with tile.TileContext(nc) as tc:
        with (
            tc.tile_pool(name="sb", bufs=1) as sb,
            tc.tile_pool(name="rot", bufs=2) as rot,
            tc.tile_pool(name="ps", bufs=2, space="PSUM") as ps,
            tc.tile_pool(name="dram", bufs=1, space="DRAM") as dr,
        ):
            # ---------- index uploads (first: gathers depend on them; idxu
            # first so the user-emb gather wins the DMA engines) ----------
            idxu = sb.tile([128, T], I32)
            nc.sync.dma_start(out=idxu[:], in_=idx_u[:])
            idxh = sb.tile([128, T * H], I32)
            nc.sync.dma_start(out=idxh[:], in_=idx_hist[:])
            idxi = sb.tile([128, T], I32)
            nc.sync.dma_start(out=idxi[:], in_=idx_i[:])

            # ---------- batched gathers (one indirect DMA per tile) ----------
            hists = []
            # user embeddings first (prod needs them), then tile-0 history in
            # two halves so attention starts on the first half ASAP
            uembs = sb.tile([128, T * D], BF)
            nc.gpsimd.indirect_dma_start(
                out=uembs[:], out_offset=None, in_=user_tab[:],
                in_offset=bass.IndirectOffsetOnAxis(ap=idxu[:], axis=1))
            hist0 = rot.tile([128, H * D], BF, tag="hist", bufs=3)
            nc.gpsimd.indirect_dma_start(
                out=hist0[:, 0:50 * D], out_offset=None, in_=user_tab[:],
                in_offset=bass.IndirectOffsetOnAxis(ap=idxh[:, 0:50], axis=1))
            nc.gpsimd.indirect_dma_start(
                out=hist0[:, 50 * D:], out_offset=None, in_=user_tab[:],
                in_offset=bass.IndirectOffsetOnAxis(ap=idxh[:, 50:H], axis=1))
            hists.append(hist0)

            iembs = sb.tile([128, T * D], BF)
            nc.gpsimd.indirect_dma_start(
                out=iembs[:], out_offset=None, in_=item_tab[:],
                in_offset=bass.IndirectOffsetOnAxis(ap=idxi[:], axis=1))

            for t in range(1, T):
                ht = rot.tile([128, H * D], BF, tag="hist", bufs=3)
                nc.gpsimd.indirect_dma_start(
                    out=ht[:], out_offset=None, in_=user_tab[:],
                    in_offset=bass.IndirectOffsetOnAxis(
                        ap=idxh[:, t * H:(t + 1) * H], axis=1))
                hists.append(ht)

            # ---------- weight / scalar uploads ----------
            # Small scalars first (DVE's first ops depend on a1s/a2s; SP is
            # in-order, so nothing may queue behind the WAW gates below).
            w3 = sb.tile([128, 4], BF)
            nc.sync.dma_start(out=w3[:], in_=w3d[:])
            g1 = sb.tile([128, 8], F32)
            nc.sync.dma_start(out=g1[:], in_=g1d[:])
            be1 = sb.tile([128, 8], F32)
            nc.sync.dma_start(out=be1[:], in_=be1d[:])
            g2 = sb.tile([128, 4], F32)
            nc.sync.dma_start(out=g2[:], in_=g2d[:])
            be2 = sb.tile([128, 4], F32)
            nc.sync.dma_start(out=be2[:], in_=be2d[:])
            a1s = sb.tile([128, 1], F32)
            nc.sync.dma_start(out=a1s[:], in_=a1d[:])
            a2s = sb.tile([128, 1], F32)
            nc.sync.dma_start(out=a2s[:], in_=a2d[:])
            b3s = sb.tile([1, 1], F32)
            nc.sync.dma_start(out=b3s[:], in_=b3d[:])
            # WAW-gate the big weight loads behind the first history gather so
            # they don't grab the DMA engines first (attention starts earlier).
            # (Program order alone is not enough: the tile scheduler reorders.)
            w1 = sb.tile([128, 3 * D1], BF)
            nc.sync.dma_start(out=w1[:, 0:1], in_=hists[0][:, 0:1])
            nc.sync.dma_start(out=w1[:], in_=w1d[:])
            w2 = sb.tile([128, 8 * D2], BF)
            nc.sync.dma_start(out=w2[:, 0:1], in_=hists[0][:, 0:1])
            nc.sync.dma_start(out=w2[:], in_=w2d[:])

            # ---------- constants ----------
            ident = sb.tile([128, 128], BF)
            make_identity(nc, ident[:])
            ones_bf = sb.tile([128, 1], BF)       # 1.0
            nc.gpsimd.memset(ones_bf[:], 1.0)
            ones_d1 = sb.tile([128, 1], BF)       # 1/1024
            nc.gpsimd.memset(ones_d1[:], 1.0 / D1)
            ones_d2 = sb.tile([128, 1], BF)       # 1/512
            nc.gpsimd.memset(ones_d2[:], 1.0 / D2)
            onesrow_bf = sb.tile([1, 128], BF)
            nc.gpsimd.memset(onesrow_bf[:], 1.0)
            ones_f32c = sb.tile([128, 1], F32)
            nc.gpsimd.memset(ones_f32c[:], 1.0)
            eps_bn = sb.tile([128, 1], F32)
            nc.gpsimd.memset(eps_bn[:], BN_EPS)
            epsd1_row = sb.tile([1, 1], F32)
            nc.gpsimd.memset(epsd1_row[:], DICE_EPS * D1)
            epsd2_row = sb.tile([1, 1], F32)
            nc.gpsimd.memset(epsd2_row[:], DICE_EPS * D2)

            # pre-warm the sqrt act table during attention (Copy/Square are
            # in the same set, so no further load until the first Sigmoid)
            warm = sb.tile([1, 1], F32)
            nc.scalar.activation(out=warm[:], in_=epsd1_row[:], func=AF.Sqrt)

            # dice alpha scalars
            oma1 = sb.tile([128, 1], F32)  # 1 - a1
            nc.vector.tensor_scalar(out=oma1[:], in0=a1s[:], scalar1=-1.0, scalar2=1.0,
                                    op0=MUL, op1=ADD)
            oma2 = sb.tile([128, 1], F32)
            nc.vector.tensor_scalar(out=oma2[:], in0=a2s[:], scalar1=-1.0, scalar2=1.0,
                                    op0=MUL, op1=ADD)

            xT = sb.tile([128, 3 * BL], BF)      # [feat-part, 3 k-chunks x 512 samples]

            # u/i transposes into xT k=0 (item), k=2 (user)
            for t in range(T):
                for k, src in ((0, iembs), (2, uembs)):
                    tp = ps.tile([128, 128], BF, tag="tp")
                    nc.tensor.transpose(out=tp[:], in_=src[:, t * D:(t + 1) * D],
                                        identity=ident[:])
                    nc.scalar.activation(out=xT[:, k * BL + t * 128:k * BL + (t + 1) * 128],
                                         in_=tp[:], func=AF.Copy)

            # layer-1 state (filled per tile inside the attention loop)
            z1T = sb.tile([128, 8 * D2], BF)    # 8 chunks x [128, 512]
            z1sq = sb.tile([128, 8 * D2], BF)   # z1^2, reused for dice1 var
            stat1p = sb.tile([128, 64], F32)    # per (m,t) partials: mean | sq
            stat1 = sb.tile([128, 16], F32)

            # ---------- attention phase (per sample-tile, all bf16 on DVE) ----------
            for t in range(T):
                hist = hists[t]
                ub = uembs[:, t * D:(t + 1) * D]
                ubb = ub.rearrange("p (o d) -> p o d", o=1).to_broadcast([128, H, D])

                prod = rot.tile([128, H * D], BF, tag="prod", bufs=1)
                h3, p3 = _r3(hist[:]), _r3(prod[:])
                if t == 0:
                    # halves: start on the first half-gather immediately
                    for lo, hi in ((0, 50), (50, H)):
                        nc.vector.tensor_tensor(out=p3[:, lo:hi], in0=h3[:, lo:hi],
                                                in1=ubb[:, lo:hi], op=MUL)
                        w = D // 2
                        while w >= 1:
                            nc.vector.tensor_tensor(
                                out=p3[:, lo:hi, 0:w], in0=p3[:, lo:hi, 0:w],
                                in1=p3[:, lo:hi, w:2 * w], op=ADD)
                            w //= 2
                else:
                    nc.vector.tensor_tensor(out=p3, in0=h3, in1=ubb, op=MUL)
                    # log-tree over d (in place, unit-stride last dim)
                    w = D // 2
                    while w >= 1:
                        nc.vector.tensor_tensor(out=p3[:, :, 0:w], in0=p3[:, :, 0:w],
                                                in1=p3[:, :, w:2 * w], op=ADD)
                        w //= 2

                # compact scores [128, H] f32 (tensor_scalar needs f32 scalars)
                scb = rot.tile([128, H], F32, tag="scb", bufs=2)
                nc.vector.tensor_copy(out=scb[:].rearrange("p (h o) -> p h o", o=1),
                                      in_=p3[:, :, 0:1])

                # scale: prod[:, h, :] = hist[:, h, :] * sc[:, h]  (tensor_scalar, 4x)
                for h in range(H):
                    eng = nc.gpsimd if (h % 4 == 3) else nc.vector
                    eng.tensor_scalar(
                        out=prod[:, h * D:(h + 1) * D], in0=hist[:, h * D:(h + 1) * D],
                        scalar1=scb[:, h:h + 1], scalar2=None, op0=MUL)

                # log-tree over h (unit-stride adds), result bf16
                nc.vector.tensor_tensor(out=p3[:, 0:50], in0=p3[:, 0:50], in1=p3[:, 50:100], op=ADD)
                nc.vector.tensor_tensor(out=p3[:, 0:25], in0=p3[:, 0:25], in1=p3[:, 25:50], op=ADD)
                nc.vector.tensor_tensor(out=p3[:, 0:12], in0=p3[:, 0:12], in1=p3[:, 12:24], op=ADD)
                nc.vector.tensor_tensor(out=p3[:, 0:6], in0=p3[:, 0:6], in1=p3[:, 6:12], op=ADD)
                nc.vector.tensor_tensor(out=p3[:, 0:3], in0=p3[:, 0:3], in1=p3[:, 3:6], op=ADD)
                nc.vector.tensor_tensor(out=p3[:, 0:1], in0=p3[:, 0:1], in1=p3[:, 1:2], op=ADD)
                nc.vector.tensor_tensor(out=p3[:, 0:1], in0=p3[:, 0:1], in1=p3[:, 2:3], op=ADD)
                his = rot.tile([128, D], BF, tag="his", bufs=2)
                nc.vector.tensor_tensor(out=his[:].rearrange("p (o d) -> p o d", o=1),
                                        in0=p3[:, 0:1], in1=p3[:, 24:25], op=ADD)

                # transpose his into xT k=1 columns
                tp = ps.tile([128, 128], BF, tag="tp")
                nc.tensor.transpose(out=tp[:], in_=his[:], identity=ident[:])
                nc.scalar.activation(out=xT[:, BL + t * 128:BL + (t + 1) * 128],
                                     in_=tp[:], func=AF.Copy)

                # layer-1 columns for this tile (overlaps later tiles' attention).
                # Last tile: split drains Act/DVE (DVE is idle post-attention).
                for m in range(8):
                    zp = ps.tile([128, 128], F32, tag="zps", bufs=3)
                    for j, k in enumerate((0, 2, 1)):   # his-dependent chunk last
                        nc.tensor.matmul(
                            zp[:],
                            lhsT=w1[:, k * D1 + m * 128:k * D1 + (m + 1) * 128],
                            rhs=xT[:, k * BL + t * 128:k * BL + (t + 1) * 128],
                            start=(j == 0), stop=(j == 2))
                    zsl = z1T[:, m * D2 + t * 128:m * D2 + (t + 1) * 128]
                    mean_sl = stat1p[:, m * 4 + t:m * 4 + t + 1]
                    sq_slot = stat1p[:, 32 + m * 4 + t:33 + m * 4 + t]
                    sqsl = z1sq[:, m * D2 + t * 128:m * D2 + (t + 1) * 128]
                    if t == T - 1 and m % 2 == 1:
                        nc.vector.tensor_scalar(out=zsl, in0=zp[:], scalar1=1.0,
                                                scalar2=None, op0=MUL,
                                                accum_out=mean_sl)
                        nc.vector.scalar_tensor_tensor(
                            out=sqsl, in0=zp[:], scalar=1.0, in1=zp[:],
                            op0=MUL, op1=MUL, accum_out=sq_slot)
                    else:
                        nc.scalar.activation(out=zsl, in_=zp[:], func=AF.Copy,
                                             accum_out=mean_sl)
                        nc.scalar.activation(out=sqsl, in_=zp[:], func=AF.Square,
                                             accum_out=sq_slot)

            # ---------- BN1 stats: split AllReduce (tiles 0-2 early, tile 3 late) ----------
            X_AX = mybir.AxisListType.X
            stat1a = sb.tile([128, 16], F32)
            nc.vector.tensor_reduce(
                out=stat1a[:, 0:8].rearrange("p (m o) -> p m o", o=1),
                in_=stat1p[:, 0:32].rearrange("p (m t) -> p m t", t=4)[:, :, 0:3],
                axis=X_AX, op=ADD)
            nc.vector.tensor_reduce(
                out=stat1a[:, 8:16].rearrange("p (m o) -> p m o", o=1),
                in_=stat1p[:, 32:64].rearrange("p (m t) -> p m t", t=4)[:, :, 0:3],
                axis=X_AX, op=ADD)
            stat1b = sb.tile([128, 16], F32)
            nc.vector.tensor_copy(
                out=stat1b[:, 0:8].rearrange("p (m o) -> p m o", o=1),
                in_=stat1p[:, 0:32].rearrange("p (m t) -> p m t", t=4)[:, :, 3:4])
            nc.vector.tensor_copy(
                out=stat1b[:, 8:16].rearrange("p (m o) -> p m o", o=1),
                in_=stat1p[:, 32:64].rearrange("p (m t) -> p m t", t=4)[:, :, 3:4])

            bi1a = dr.tile([128, 16], F32)
            bo1a = dr.tile([128, 16], F32)
            bi1b = dr.tile([128, 16], F32)
            bo1b = dr.tile([128, 16], F32)
            nc.sync.dma_start(out=bi1a[:], in_=stat1a[:])
            nc.sync.dma_start(out=bi1b[:], in_=stat1b[:])
            if sim_mode:
                nc.gpsimd.dma_start(out=bo1a[:], in_=bi1a[:])
                nc.gpsimd.dma_start(out=bo1b[:], in_=bi1b[:])
            else:
                nc.gpsimd.collective_compute(
                    "AllReduce", ADD, replica_groups=[list(range(NCORES))],
                    ins=[bi1a.opt()], outs=[bo1a.opt()])
                nc.gpsimd.collective_compute(
                    "AllReduce", ADD, replica_groups=[list(range(NCORES))],
                    ins=[bi1b.opt()], outs=[bo1b.opt()])
            ast1a = sb.tile([128, 16], F32)
            nc.sync.dma_start(out=ast1a[:], in_=bo1a[:])
            ast1b = sb.tile([128, 16], F32)
            nc.sync.dma_start(out=ast1b[:], in_=bo1b[:])
            nc.vector.tensor_tensor(out=stat1[:], in0=ast1a[:], in1=ast1b[:], op=ADD)

            # BN1 affine: s = g / sqrt(var+eps), t = be - mu*s
            mu1 = sb.tile([128, 8], F32)
            nc.vector.tensor_scalar(out=mu1[:], in0=stat1[:, 0:8], scalar1=1.0 / B,
                                    scalar2=None, op0=MUL)
            var1 = sb.tile([128, 8], F32)
            nc.vector.tensor_scalar(out=var1[:], in0=stat1[:, 8:16], scalar1=1.0 / B,
                                    scalar2=None, op0=MUL)
            musq1 = sb.tile([128, 8], F32)
            nc.vector.tensor_tensor(out=musq1[:], in0=mu1[:], in1=mu1[:], op=MUL)
            nc.vector.tensor_tensor(out=var1[:], in0=var1[:], in1=musq1[:], op=SUB)
            sd1 = sb.tile([128, 8], F32)
            nc.scalar.activation(out=sd1[:], in_=var1[:], func=AF.Sqrt, bias=eps_bn[:])
            inv1 = sb.tile([128, 8], F32)
            nc.vector.reciprocal(out=inv1[:], in_=sd1[:])
            s1 = sb.tile([128, 8], F32)
            nc.vector.tensor_tensor(out=s1[:], in0=g1[:], in1=inv1[:], op=MUL)
            t1 = sb.tile([128, 8], F32)
            nc.vector.tensor_tensor(out=t1[:], in0=mu1[:], in1=s1[:], op=MUL)
            nc.vector.tensor_tensor(out=t1[:], in0=be1[:], in1=t1[:], op=SUB)
            s1sq = sb.tile([128, 8], F32)
            nc.vector.tensor_tensor(out=s1sq[:], in0=s1[:], in1=s1[:], op=MUL)
            st1 = sb.tile([128, 8], F32)   # 2*s*t
            nc.vector.tensor_tensor(out=st1[:], in0=s1[:], in1=t1[:], op=MUL)
            nc.vector.tensor_scalar(out=st1[:], in0=st1[:], scalar1=2.0,
                                    scalar2=None, op0=MUL)
            tsq1 = sb.tile([128, 8], F32)
            nc.vector.tensor_tensor(out=tsq1[:], in0=t1[:], in1=t1[:], op=MUL)
            # sum(t^2) over all features -> [1,1], folded into the Sqrt bias
            tsq1p = ps.tile([1, 8], F32, tag="cs", bufs=1)
            nc.tensor.matmul(tsq1p[:], lhsT=ones_f32c[:], rhs=tsq1[:], start=True, stop=True)
            bias_d1 = sb.tile([1, 1], F32)
            nc.vector.tensor_reduce(
                out=bias_d1[:].rearrange("p (o c) -> p o c", o=1),
                in_=tsq1p[:].rearrange("p (o c) -> p o c", o=1), axis=X_AX, op=ADD)
            nc.vector.tensor_tensor(out=bias_d1[:], in0=bias_d1[:], in1=epsd1_row[:], op=ADD)

            y1 = sb.tile([128, 8 * D2], BF)
            for m in range(8):
                nc.vector.tensor_scalar(
                    out=y1[:, m * D2:(m + 1) * D2], in0=z1T[:, m * D2:(m + 1) * D2],
                    scalar1=s1[:, m:m + 1], scalar2=t1[:, m:m + 1], op0=MUL, op1=ADD)

            # ---------- Dice 1 (feature mean/var via PE ones-matmuls, bf16) ----------
            avgp = ps.tile([1, BL], F32, tag="cs", bufs=1)
            for m in range(8):
                nc.tensor.matmul(avgp[:], lhsT=ones_d1[:], rhs=y1[:, m * D2:(m + 1) * D2],
                                 start=(m == 0), stop=(m == 7))
            avgrow = sb.tile([1, BL], BF)
            nc.vector.tensor_copy(out=avgrow[:], in_=avgp[:])
            avgb = ps.tile([128, BL], F32, tag="bc")
            nc.tensor.matmul(avgb[:], lhsT=onesrow_bf[:], rhs=avgrow[:], start=True, stop=True)
            avgb_sb = sb.tile([128, BL], BF)
            nc.scalar.activation(out=avgb_sb[:], in_=avgb[:], func=AF.Copy)

            diff1 = sb.tile([128, 8 * D2], BF)
            for m in range(8):
                nc.vector.tensor_tensor(out=diff1[:, m * D2:(m + 1) * D2],
                                        in0=y1[:, m * D2:(m + 1) * D2],
                                        in1=avgb_sb[:], op=SUB)

            # var per sample = sum(y^2) - D1*avg^2 (+ eps*D1 + sum(t^2) in bias)
            # with sum(y^2) = sum(s^2 z^2) + sum(2st z) over the z drains
            s1sq_b = sb.tile([128, 8], BF)
            nc.vector.tensor_copy(out=s1sq_b[:], in_=s1sq[:])
            st1_b = sb.tile([128, 8], BF)
            nc.vector.tensor_copy(out=st1_b[:], in_=st1[:])
            varp = ps.tile([1, BL], F32, tag="cs", bufs=1)
            for m in range(8):
                nc.tensor.matmul(varp[:], lhsT=s1sq_b[:, m:m + 1],
                                 rhs=z1sq[:, m * D2:(m + 1) * D2],
                                 start=(m == 0), stop=False)
                nc.tensor.matmul(varp[:], lhsT=st1_b[:, m:m + 1],
                                 rhs=z1T[:, m * D2:(m + 1) * D2],
                                 start=False, stop=(m == 7))
            avgsq = sb.tile([1, BL], F32)
            nc.vector.tensor_tensor(out=avgsq[:], in0=avgrow[:], in1=avgrow[:], op=MUL)
            vrow = sb.tile([1, BL], F32)
            nc.vector.scalar_tensor_tensor(out=vrow[:], in0=avgsq[:], scalar=-float(D1),
                                           in1=varp[:], op0=MUL, op1=ADD)
            sqrow = sb.tile([1, BL], F32)
            nc.scalar.activation(out=sqrow[:], in_=vrow[:], func=AF.Sqrt, bias=bias_d1[:])
            rstd = sb.tile([1, BL], F32)
            nc.vector.reciprocal(out=rstd[:], in_=sqrow[:])
            rstdb16 = sb.tile([1, BL], BF)
            nc.vector.tensor_copy(out=rstdb16[:], in_=rstd[:])
            rstdb = ps.tile([128, BL], F32, tag="bc")
            nc.tensor.matmul(rstdb[:], lhsT=onesrow_bf[:], rhs=rstdb16[:], start=True, stop=True)
            rstdb_sb = sb.tile([128, BL], BF)
            nc.scalar.activation(out=rstdb_sb[:], in_=rstdb[:], func=AF.Copy)

            for m in range(8):
                sl = slice(m * D2, (m + 1) * D2)
                nc.vector.tensor_tensor(out=diff1[:, sl], in0=diff1[:, sl], in1=rstdb_sb[:], op=MUL)
                nc.scalar.activation(out=diff1[:, sl], in_=diff1[:, sl], func=AF.Sigmoid)
                nc.vector.tensor_scalar(out=diff1[:, sl], in0=diff1[:, sl],
                                        scalar1=oma1[:], scalar2=a1s[:], op0=MUL, op1=ADD)
                nc.vector.tensor_tensor(out=y1[:, sl], in0=y1[:, sl], in1=diff1[:, sl], op=MUL)

            # ---------- layer 2 ----------
            z2T = sb.tile([128, 4 * D2], BF)
            z2sq = sb.tile([128, 4 * D2], BF)
            stat2 = sb.tile([128, 8], F32)
            for m in range(4):
                zp = ps.tile([128, D2], F32, tag="zps", bufs=3)
                for k in range(8):
                    nc.tensor.matmul(zp[:], lhsT=w2[:, k * D2 + m * 128:k * D2 + (m + 1) * 128],
                                     rhs=y1[:, k * D2:(k + 1) * D2],
                                     start=(k == 0), stop=(k == 7))
                nc.scalar.activation(out=z2T[:, m * D2:(m + 1) * D2], in_=zp[:], func=AF.Copy,
                                     accum_out=stat2[:, m:m + 1])
                nc.scalar.activation(out=z2sq[:, m * D2:(m + 1) * D2], in_=zp[:], func=AF.Square,
                                     accum_out=stat2[:, 4 + m:5 + m])

            bi2 = dr.tile([128, 8], F32)
            bo2 = dr.tile([128, 8], F32)
            nc.sync.dma_start(out=bi2[:], in_=stat2[:])
            if sim_mode:
                nc.gpsimd.dma_start(out=bo2[:], in_=bi2[:])
            else:
                nc.gpsimd.collective_compute(
                    "AllReduce", ADD, replica_groups=[list(range(NCORES))],
                    ins=[bi2.opt()], outs=[bo2.opt()])
            ast2 = sb.tile([128, 8], F32)
            nc.sync.dma_start(out=ast2[:], in_=bo2[:])

            mu2 = sb.tile([128, 4], F32)
            nc.vector.tensor_scalar(out=mu2[:], in0=ast2[:, 0:4], scalar1=1.0 / B,
                                    scalar2=None, op0=MUL)
            var2 = sb.tile([128, 4], F32)
            nc.vector.tensor_scalar(out=var2[:], in0=ast2[:, 4:8], scalar1=1.0 / B,
                                    scalar2=None, op0=MUL)
            musq2 = sb.tile([128, 4], F32)
            nc.vector.tensor_tensor(out=musq2[:], in0=mu2[:], in1=mu2[:], op=MUL)
            nc.vector.tensor_tensor(out=var2[:], in0=var2[:], in1=musq2[:], op=SUB)
            sd2 = sb.tile([128, 4], F32)
            nc.scalar.activation(out=sd2[:], in_=var2[:], func=AF.Sqrt, bias=eps_bn[:])
            inv2 = sb.tile([128, 4], F32)
            nc.vector.reciprocal(out=inv2[:], in_=sd2[:])
            s2 = sb.tile([128, 4], F32)
            nc.vector.tensor_tensor(out=s2[:], in0=g2[:], in1=inv2[:], op=MUL)
            t2 = sb.tile([128, 4], F32)
            nc.vector.tensor_tensor(out=t2[:], in0=mu2[:], in1=s2[:], op=MUL)
            nc.vector.tensor_tensor(out=t2[:], in0=be2[:], in1=t2[:], op=SUB)
            s2sq = sb.tile([128, 4], F32)
            nc.vector.tensor_tensor(out=s2sq[:], in0=s2[:], in1=s2[:], op=MUL)
            st2 = sb.tile([128, 4], F32)
            nc.vector.tensor_tensor(out=st2[:], in0=s2[:], in1=t2[:], op=MUL)
            nc.vector.tensor_scalar(out=st2[:], in0=st2[:], scalar1=2.0,
                                    scalar2=None, op0=MUL)
            tsq2 = sb.tile([128, 4], F32)
            nc.vector.tensor_tensor(out=tsq2[:], in0=t2[:], in1=t2[:], op=MUL)
            tsq2p = ps.tile([1, 4], F32, tag="cs", bufs=1)
            nc.tensor.matmul(tsq2p[:], lhsT=ones_f32c[:], rhs=tsq2[:], start=True, stop=True)
            bias_d2 = sb.tile([1, 1], F32)
            nc.vector.tensor_reduce(
                out=bias_d2[:].rearrange("p (o c) -> p o c", o=1),
                in_=tsq2p[:].rearrange("p (o c) -> p o c", o=1), axis=X_AX, op=ADD)
            nc.vector.tensor_tensor(out=bias_d2[:], in0=bias_d2[:], in1=epsd2_row[:], op=ADD)
            s2sq_b = sb.tile([128, 4], BF)
            nc.vector.tensor_copy(out=s2sq_b[:], in_=s2sq[:])
            st2_b = sb.tile([128, 4], BF)
            nc.vector.tensor_copy(out=st2_b[:], in_=st2[:])

            y2 = sb.tile([128, 4 * D2], BF)   # keep raw z2T for the var matmuls
            for m in range(4):
                sl = slice(m * D2, (m + 1) * D2)
                nc.vector.tensor_scalar(
                    out=y2[:, sl], in0=z2T[:, sl],
                    scalar1=s2[:, m:m + 1], scalar2=t2[:, m:m + 1], op0=MUL, op1=ADD)

            # ---------- Dice 2 (bf16) ----------
            avgp2 = ps.tile([1, BL], F32, tag="cs", bufs=1)
            for m in range(4):
                nc.tensor.matmul(avgp2[:], lhsT=ones_d2[:], rhs=y2[:, m * D2:(m + 1) * D2],
                                 start=(m == 0), stop=(m == 3))
            avgrow2 = sb.tile([1, BL], BF)
            nc.vector.tensor_copy(out=avgrow2[:], in_=avgp2[:])
            avgb2 = ps.tile([128, BL], F32, tag="bc")
            nc.tensor.matmul(avgb2[:], lhsT=onesrow_bf[:], rhs=avgrow2[:], start=True, stop=True)
            avgb2_sb = sb.tile([128, BL], BF)
            nc.scalar.activation(out=avgb2_sb[:], in_=avgb2[:], func=AF.Copy)

            diff2 = sb.tile([128, 4 * D2], BF)
            for m in range(4):
                nc.vector.tensor_tensor(out=diff2[:, m * D2:(m + 1) * D2],
                                        in0=y2[:, m * D2:(m + 1) * D2], in1=avgb2_sb[:], op=SUB)
            varp2 = ps.tile([1, BL], F32, tag="cs", bufs=1)
            for m in range(4):
                nc.tensor.matmul(varp2[:], lhsT=s2sq_b[:, m:m + 1],
                                 rhs=z2sq[:, m * D2:(m + 1) * D2],
                                 start=(m == 0), stop=False)
                nc.tensor.matmul(varp2[:], lhsT=st2_b[:, m:m + 1],
                                 rhs=z2T[:, m * D2:(m + 1) * D2],
                                 start=False, stop=(m == 3))
            avgsq2 = sb.tile([1, BL], F32)
            nc.vector.tensor_tensor(out=avgsq2[:], in0=avgrow2[:], in1=avgrow2[:], op=MUL)
            vrow2 = sb.tile([1, BL], F32)
            nc.vector.scalar_tensor_tensor(out=vrow2[:], in0=avgsq2[:], scalar=-float(D2),
                                           in1=varp2[:], op0=MUL, op1=ADD)
            sqrow2 = sb.tile([1, BL], F32)
            nc.scalar.activation(out=sqrow2[:], in_=vrow2[:], func=AF.Sqrt, bias=bias_d2[:])
            rstd2 = sb.tile([1, BL], F32)
            nc.vector.reciprocal(out=rstd2[:], in_=sqrow2[:])
            rstd2b = sb.tile([1, BL], BF)
            nc.vector.tensor_copy(out=rstd2b[:], in_=rstd2[:])
            rstdb2 = ps.tile([128, BL], F32, tag="bc")
            nc.tensor.matmul(rstdb2[:], lhsT=onesrow_bf[:], rhs=rstd2b[:], start=True, stop=True)
            rstdb2_sb = sb.tile([128, BL], BF)
            nc.scalar.activation(out=rstdb2_sb[:], in_=rstdb2[:], func=AF.Copy)

            for m in range(4):
                sl = slice(m * D2, (m + 1) * D2)
                nc.vector.tensor_tensor(out=diff2[:, sl], in0=diff2[:, sl], in1=rstdb2_sb[:], op=MUL)
                nc.scalar.activation(out=diff2[:, sl], in_=diff2[:, sl], func=AF.Sigmoid)
                nc.vector.tensor_scalar(out=diff2[:, sl], in0=diff2[:, sl],
                                        scalar1=oma2[:], scalar2=a2s[:], op0=MUL, op1=ADD)
                nc.vector.tensor_tensor(out=y2[:, sl], in0=y2[:, sl], in1=diff2[:, sl], op=MUL)

            # ---------- layer 3: out row = W3^T y2 + b3 ----------
            z3p = ps.tile([1, BL], F32, tag="cs", bufs=1)
            for k in range(4):
                nc.tensor.matmul(z3p[:], lhsT=w3[:, k:k + 1], rhs=y2[:, k * D2:(k + 1) * D2],
                                 start=(k == 0), stop=(k == 3))
            z3row = sb.tile([1, BL], F32)
            nc.vector.tensor_scalar(out=z3row[:], in0=z3p[:], scalar1=b3s[0:1, 0:1],
                                    scalar2=None, op0=ADD)
            nc.sync.dma_start(out=outd[:], in_=z3row[:])

    nc.compile()
    return nc


def _get_prog():
    global _PROG
    if _PROG is None:
        _PROG = _build()
    return _PROG


def kernel(items, users, history_users, item_table, user_table,
           W1, b1, g1, be1, a1, W2, b2, g2, be2, a2, W3, b3):
    nc = _get_prog()

    items = np.asarray(items).astype(np.int32)
    users = np.asarray(users).astype(np.int32)
    hist = np.asarray(history_users).astype(np.int32)
    item_table = np.ascontiguousarray(
        np.asarray(item_table, dtype=np.float32)).astype(
            ml_dtypes.bfloat16).reshape(N_ITEM * D // 4096, 4096)
    user_table = np.ascontiguousarray(
        np.asarray(user_table, dtype=np.float32)).astype(
            ml_dtypes.bfloat16).reshape(M_USER * D // 4096, 4096)
    W1 = np.asarray(W1, dtype=np.float32)
    W2 = np.asarray(W2, dtype=np.float32)
    W3 = np.asarray(W3, dtype=np.float32)

    # host-side weight reshapes (shared across cores)
    w1sb = W1.reshape(3, 128, D1).transpose(1, 0, 2).reshape(128, 3 * D1)
    w1sb = np.ascontiguousarray(w1sb).astype(ml_dtypes.bfloat16)
    w2sb = W2.reshape(8, 128, D2).transpose(1, 0, 2).reshape(128, 8 * D2)
    w2sb = np.ascontiguousarray(w2sb).astype(ml_dtypes.bfloat16)
    w3sb = np.ascontiguousarray(W3.reshape(4, 128).T).astype(ml_dtypes.bfloat16)
    g1r = np.ascontiguousarray(np.asarray(g1, np.float32).reshape(8, 128).T)
    be1r = np.ascontiguousarray(np.asarray(be1, np.float32).reshape(8, 128).T)
    g2r = np.ascontiguousarray(np.asarray(g2, np.float32).reshape(4, 128).T)
    be2r = np.ascontiguousarray(np.asarray(be2, np.float32).reshape(4, 128).T)
    a1c = np.full((128, 1), np.float32(np.asarray(a1).ravel()[0]), np.float32)
    a2c = np.full((128, 1), np.float32(np.asarray(a2).ravel()[0]), np.float32)
    b3c = np.full((1, 1), np.float32(np.asarray(b3).ravel()[0]), np.float32)

    in_maps = []
    for c in range(NCORES):
        sl = slice(c * BL, (c + 1) * BL)
        idx_hist = (hist[sl] * D).reshape(T, 128, H).transpose(1, 0, 2).reshape(128, T * H)
        idx_u = (users[sl] * D).reshape(T, 128).T
        idx_i = (items[sl] * D).reshape(T, 128).T
        in_maps.append({
            "user_table": user_table, "item_table": item_table,
            "idx_hist": np.ascontiguousarray(idx_hist),
            "idx_u": np.ascontiguousarray(idx_u),
            "idx_i": np.ascontiguousarray(idx_i),
            "w1sb": w1sb, "w2sb": w2sb, "w3sb": w3sb,
            "g1r": g1r, "be1r": be1r, "g2r": g2r, "be2r": be2r,
            "a1c": a1c, "a2c": a2c, "b3c": b3c,
        })

    res = run_bass_kernel_spmd(nc, in_maps, core_ids=list(range(NCORES)))
    out = np.concatenate(
        [np.asarray(res.results[c]["out"], np.float32).reshape(BL, 1) for c in range(NCORES)],
        axis=0)
    return out


# revision 29
# speedup vs baseline: 1.0230x; 1.0230x over previous
"""DIN (attention pooling + MLP w/ BatchNorm+Dice) Trainium2 kernel, 8-core SPMD.

Contract: kernel(**inputs) takes FULL unsharded inputs (numpy), returns [4096,1] f32.
Internally: batch-sharded 512 samples/core; embedding tables + weights replicated.

Tables are uploaded bf16; history embeddings for a 128-sample tile are fetched
with a single batched indirect DMA (offset AP [128,100]); attention runs on DVE
in bf16 (product w/ broadcast, in-place log-tree over d, per-h tensor_scalar
scale, log-tree over h).
"""
import sys
sys.path.insert(0, "/opt/trn_rl_repo")
import numpy as np
import ml_dtypes

import concourse.bass as bass
import concourse.mybir as mybir
import concourse.tile as tile
from concourse import bacc
from concourse.bass_utils import run_bass_kernel_spmd
from concourse.masks import make_identity

B, H, D = 4096, 100, 128
N_ITEM, M_USER = 100000, 500000
D1, D2 = 1024, 512
NCORES = 8
BL = B // NCORES          # 512 samples per core
T = BL // 128             # 4 sample-tiles per core
DICE_EPS, BN_EPS = 1e-3, 1e-5

BF = mybir.dt.bfloat16
F32 = mybir.dt.float32
I32 = mybir.dt.int32
MUL = mybir.AluOpType.mult
ADD = mybir.AluOpType.add
SUB = mybir.AluOpType.subtract
AF = mybir.ActivationFunctionType

_PROG = None


def _r3(t_ap):
    return t_ap.rearrange("p (h d) -> p h d", d=D)


def _bcol(col_ap, n):
    """[128,1] AP -> [128,n] broadcast AP (step-0 inner dim)."""
    return col_ap.to_broadcast([col_ap.shape[0], n])


def _build(sim_mode=False):
    ndev = 1 if sim_mode else NCORES
    nc = bacc.Bacc("TRN2", target_bir_lowering=False, debug=False, num_devices=ndev)

    # Tables are declared with a wide inner dim (same row-major bytes); gather
    # indices are pre-scaled by D on the host and used with axis=1 (coef=1),
    # so each index is a flat element offset into the table.
    user_tab = nc.dram_tensor("user_table", [M_USER * D // 4096, 4096], BF,
                              kind="ExternalInput")
    item_tab = nc.dram_tensor("item_table", [N_ITEM * D // 4096, 4096], BF,
                              kind="ExternalInput")
    idx_hist = nc.dram_tensor("idx_hist", [128, T * H], I32, kind="ExternalInput")
    idx_u = nc.dram_tensor("idx_u", [128, T], I32, kind="ExternalInput")
    idx_i = nc.dram_tensor("idx_i", [128, T], I32, kind="ExternalInput")
    w1d = nc.dram_tensor("w1sb", [128, 3 * D1], BF, kind="ExternalInput")
    w2d = nc.dram_tensor("w2sb", [128, 8 * D2], BF, kind="ExternalInput")
    w3d = nc.dram_tensor("w3sb", [128, 4], BF, kind="ExternalInput")
    g1d = nc.dram_tensor("g1r", [128, 8], F32, kind="ExternalInput")
    be1d = nc.dram_tensor("be1r", [128, 8], F32, kind="ExternalInput")
    g2d = nc.dram_tensor("g2r", [128, 4], F32, kind="ExternalInput")
    be2d = nc.dram_tensor("be2r", [128, 4], F32, kind="ExternalInput")
    a1d = nc.dram_tensor("a1c", [128, 1], F32, kind="ExternalInput")
    a2d = nc.dram_tensor("a2c", [128, 1], F32, kind="ExternalInput")
    b3d = nc.dram_tensor("b3c", [1, 1], F32, kind="ExternalInput")
    outd = nc.dram_tensor("out", [1, BL], F32, kind="ExternalOutput")

    with tile.TileContext(nc) as tc:
        with (
            tc.tile_pool(name="sb", bufs=1) as sb,
            tc.tile_pool(name="rot", bufs=2) as rot,
            tc.tile_pool(name="ps", bufs=2, space="PSUM") as ps,
            tc.tile_pool(name="dram", bufs=1, space="DRAM") as dr,
        ):
            # ---------- index uploads (first: gathers depend on them; idxu
            # first so the user-emb gather wins the DMA engines) ----------
            idxu = sb.tile([128, T], I32)
            nc.sync.dma_start(out=idxu[:], in_=idx_u[:])
            idxh = sb.tile([128, T * H], I32)
            nc.sync.dma_start(out=idxh[:], in_=idx_hist[:])
            idxi = sb.tile([128, T], I32)
            nc.sync.dma_start(out=idxi[:], in_=idx_i[:])

            # ---------- batched gathers (one indirect DMA per tile) ----------
            hists = []
            # user embeddings first (prod needs them), then tile-0 history in
            # two halves so attention starts on the first half ASAP
            uembs = sb.tile([128, T * D], BF)
            nc.gpsimd.indirect_dma_start(
                out=uembs[:], out_offset=None, in_=user_tab[:],
                in_offset=bass.IndirectOffsetOnAxis(ap=idxu[:], axis=1))
            hist0 = rot.tile([128, H * D], BF, tag="hist", bufs=3)
            nc.gpsimd.indirect_dma_start(
                out=hist0[:, 0:50 * D], out_offset=None, in_=user_tab[:],
                in_offset=bass.IndirectOffsetOnAxis(ap=idxh[:, 0:50], axis=1))
            nc.gpsimd.indirect_dma_start(
                out=hist0[:, 50 * D:], out_offset=None, in_=user_tab[:],
                in_offset=bass.IndirectOffsetOnAxis(ap=idxh[:, 50:H], axis=1))
            hists.append(hist0)

            iembs = sb.tile([128, T * D], BF)
            nc.gpsimd.indirect_dma_start(
                out=iembs[:], out_offset=None, in_=item_tab[:],
                in_offset=bass.IndirectOffsetOnAxis(ap=idxi[:], axis=1))

            for t in range(1, T):
                ht = rot.tile([128, H * D], BF, tag="hist", bufs=3)
                nc.gpsimd.indirect_dma_start(
                    out=ht[:], out_offset=None, in_=user_tab[:],
                    in_offset=bass.IndirectOffsetOnAxis(
                        ap=idxh[:, t * H:(t + 1) * H], axis=1))
                hists.append(ht)

            # ---------- weight / scalar uploads ----------
            # Small scalars first (DVE's first ops depend on a1s/a2s; SP is
            # in-order, so nothing may queue behind the WAW gates below).
            a1s = sb.tile([128, 1], F32)
            nc.sync.dma_start(out=a1s[:], in_=a1d[:])
            a2s = sb.tile([128, 1], F32)
            nc.sync.dma_start(out=a2s[:], in_=a2d[:])
            w3 = sb.tile([128, 4], BF)
            nc.sync.dma_start(out=w3[:], in_=w3d[:])
            g1 = sb.tile([128, 8], F32)
            nc.sync.dma_start(out=g1[:], in_=g1d[:])
            be1 = sb.tile([128, 8], F32)
            nc.sync.dma_start(out=be1[:], in_=be1d[:])
            g2 = sb.tile([128, 4], F32)
            nc.sync.dma_start(out=g2[:], in_=g2d[:])
            be2 = sb.tile([128, 4], F32)
            nc.sync.dma_start(out=be2[:], in_=be2d[:])
            b3s = sb.tile([1, 1], F32)
            nc.sync.dma_start(out=b3s[:], in_=b3d[:])
            # WAW-gate the big weight loads behind the first history gather so
            # they don't grab the DMA engines first (attention starts earlier).
            # (Program order alone is not enough: the tile scheduler reorders.)
            w1 = sb.tile([128, 3 * D1], BF)
            nc.sync.dma_start(out=w1[:, 0:1], in_=hists[0][:, 0:1])
            nc.sync.dma_start(out=w1[:], in_=w1d[:])
            w2 = sb.tile([128, 8 * D2], BF)
            nc.sync.dma_start(out=w2[:, 0:1], in_=hists[0][:, 0:1])
            nc.sync.dma_start(out=w2[:], in_=w2d[:])

            # ---------- constants ----------
            ident = sb.tile([128, 128], BF)
            make_identity(nc, ident[:])
            ones_bf = sb.tile([128, 1], BF)       # 1.0
            nc.gpsimd.memset(ones_bf[:], 1.0)
            ones_d1 = sb.tile([128, 1], BF)       # 1/1024
            nc.gpsimd.memset(ones_d1[:], 1.0 / D1)
            ones_d2 = sb.tile([128, 1], BF)       # 1/512
            nc.gpsimd.memset(ones_d2[:], 1.0 / D2)
            onesrow_bf = sb.tile([1, 128], BF)
            nc.gpsimd.memset(onesrow_bf[:], 1.0)
            ones_f32c = sb.tile([128, 1], F32)
            nc.gpsimd.memset(ones_f32c[:], 1.0)
            eps_bn = sb.tile([128, 1], F32)
            nc.gpsimd.memset(eps_bn[:], BN_EPS)
            epsd1_row = sb.tile([1, 1], F32)
            nc.gpsimd.memset(epsd1_row[:], DICE_EPS * D1)
            epsd2_row = sb.tile([1, 1], F32)
            nc.gpsimd.memset(epsd2_row[:], DICE_EPS * D2)

            # pre-warm the sqrt act table during attention (Copy/Square are
            # in the same set, so no further load until the first Sigmoid)
            warm = sb.tile([1, 1], F32)
            nc.scalar.activation(out=warm[:], in_=epsd1_row[:], func=AF.Sqrt)

            xT = sb.tile([128, 3 * BL], BF)      # [feat-part, 3 k-chunks x 512 samples]

            # u/i transposes into xT k=0 (item), k=2 (user)
            for t in range(T):
                for k, src in ((0, iembs), (2, uembs)):
                    tp = ps.tile([128, 128], BF, tag="tp")
                    nc.tensor.transpose(out=tp[:], in_=src[:, t * D:(t + 1) * D],
                                        identity=ident[:])
                    nc.scalar.activation(out=xT[:, k * BL + t * 128:k * BL + (t + 1) * 128],
                                         in_=tp[:], func=AF.Copy)

            # layer-1 state (filled per tile inside the attention loop)
            z1T = sb.tile([128, 8 * D2], BF)    # 8 chunks x [128, 512]
            z1sq = sb.tile([128, 8 * D2], BF)   # z1^2, reused for dice1 var
            stat1p = sb.tile([128, 64], F32)    # per (m,t) partials: mean | sq
            stat1 = sb.tile([128, 16], F32)

            # ---------- attention phase (per sample-tile, all bf16 on DVE) ----------
            for t in range(T):
                hist = hists[t]
                ub = uembs[:, t * D:(t + 1) * D]
                ubb = ub.rearrange("p (o d) -> p o d", o=1).to_broadcast([128, H, D])

                prod = rot.tile([128, H * D], BF, tag="prod", bufs=1)
                h3, p3 = _r3(hist[:]), _r3(prod[:])
                if t == 0:
                    # halves: start on the first half-gather immediately
                    for lo, hi in ((0, 50), (50, H)):
                        nc.vector.tensor_tensor(out=p3[:, lo:hi], in0=h3[:, lo:hi],
                                                in1=ubb[:, lo:hi], op=MUL)
                        w = D // 2
                        while w >= 1:
                            nc.vector.tensor_tensor(
                                out=p3[:, lo:hi, 0:w], in0=p3[:, lo:hi, 0:w],
                                in1=p3[:, lo:hi, w:2 * w], op=ADD)
                            w //= 2
                else:
                    nc.vector.tensor_tensor(out=p3, in0=h3, in1=ubb, op=MUL)
                    # log-tree over d (in place, unit-stride last dim)
                    w = D // 2
                    while w >= 1:
                        nc.vector.tensor_tensor(out=p3[:, :, 0:w], in0=p3[:, :, 0:w],
                                                in1=p3[:, :, w:2 * w], op=ADD)
                        w //= 2

                # compact scores [128, H] f32 (tensor_scalar needs f32 scalars)
                scb = rot.tile([128, H], F32, tag="scb", bufs=2)
                nc.vector.tensor_copy(out=scb[:].rearrange("p (h o) -> p h o", o=1),
                                      in_=p3[:, :, 0:1])

                # scale: prod[:, h, :] = hist[:, h, :] * sc[:, h]  (tensor_scalar, 4x)
                for h in range(H):
                    eng = nc.gpsimd if (h % 4 == 3) else nc.vector
                    eng.tensor_scalar(
                        out=prod[:, h * D:(h + 1) * D], in0=hist[:, h * D:(h + 1) * D],
                        scalar1=scb[:, h:h + 1], scalar2=None, op0=MUL)

                # log-tree over h (unit-stride adds), result bf16
                nc.vector.tensor_tensor(out=p3[:, 0:50], in0=p3[:, 0:50], in1=p3[:, 50:100], op=ADD)
                nc.vector.tensor_tensor(out=p3[:, 0:25], in0=p3[:, 0:25], in1=p3[:, 25:50], op=ADD)
                nc.vector.tensor_tensor(out=p3[:, 0:12], in0=p3[:, 0:12], in1=p3[:, 12:24], op=ADD)
                nc.vector.tensor_tensor(out=p3[:, 0:6], in0=p3[:, 0:6], in1=p3[:, 6:12], op=ADD)
                nc.vector.tensor_tensor(out=p3[:, 0:3], in0=p3[:, 0:3], in1=p3[:, 3:6], op=ADD)
                nc.vector.tensor_tensor(out=p3[:, 0:1], in0=p3[:, 0:1], in1=p3[:, 1:2], op=ADD)
                nc.vector.tensor_tensor(out=p3[:, 0:1], in0=p3[:, 0:1], in1=p3[:, 2:3], op=ADD)
                his = rot.tile([128, D], BF, tag="his", bufs=2)
                nc.vector.tensor_tensor(out=his[:].rearrange("p (o d) -> p o d", o=1),
                                        in0=p3[:, 0:1], in1=p3[:, 24:25], op=ADD)

                # transpose his into xT k=1 columns
                tp = ps.tile([128, 128], BF, tag="tp")
                nc.tensor.transpose(out=tp[:], in_=his[:], identity=ident[:])
                nc.scalar.activation(out=xT[:, BL + t * 128:BL + (t + 1) * 128],
                                     in_=tp[:], func=AF.Copy)

                # layer-1 columns for this tile (overlaps later tiles' attention).
                # Last tile: split drains Act/DVE (DVE is idle post-attention).
                for m in range(8):
                    zp = ps.tile([128, 128], F32, tag="zps", bufs=3)
                    for j, k in enumerate((0, 2, 1)):   # his-dependent chunk last
                        nc.tensor.matmul(
                            zp[:],
                            lhsT=w1[:, k * D1 + m * 128:k * D1 + (m + 1) * 128],
                            rhs=xT[:, k * BL + t * 128:k * BL + (t + 1) * 128],
                            start=(j == 0), stop=(j == 2))
                    zsl = z1T[:, m * D2 + t * 128:m * D2 + (t + 1) * 128]
                    mean_sl = stat1p[:, m * 4 + t:m * 4 + t + 1]
                    sq_slot = stat1p[:, 32 + m * 4 + t:33 + m * 4 + t]
                    sqsl = z1sq[:, m * D2 + t * 128:m * D2 + (t + 1) * 128]
                    if t == T - 1 and m % 2 == 1:
                        nc.vector.tensor_scalar(out=zsl, in0=zp[:], scalar1=1.0,
                                                scalar2=None, op0=MUL,
                                                accum_out=mean_sl)
                        nc.vector.scalar_tensor_tensor(
                            out=sqsl, in0=zp[:], scalar=1.0, in1=zp[:],
                            op0=MUL, op1=MUL, accum_out=sq_slot)
                    else:
                        nc.scalar.activation(out=zsl, in_=zp[:], func=AF.Copy,
                                             accum_out=mean_sl)
                        nc.scalar.activation(out=sqsl, in_=zp[:], func=AF.Square,
                                             accum_out=sq_slot)

            # dice alpha scalars (emitted late: must not stall DVE's in-order
            # stream ahead of the attention ops)
            oma1 = sb.tile([128, 1], F32)  # 1 - a1
            nc.vector.tensor_scalar(out=oma1[:], in0=a1s[:], scalar1=-1.0, scalar2=1.0,
                                    op0=MUL, op1=ADD)
            oma2 = sb.tile([128, 1], F32)
            nc.vector.tensor_scalar(out=oma2[:], in0=a2s[:], scalar1=-1.0, scalar2=1.0,
                                    op0=MUL, op1=ADD)

            # ---------- BN1 stats: split AllReduce (tiles 0-2 early, tile 3 late) ----------
            X_AX = mybir.AxisListType.X
            stat1a = sb.tile([128, 16], F32)
            nc.vector.tensor_reduce(
                out=stat1a[:, 0:8].rearrange("p (m o) -> p m o", o=1),
                in_=stat1p[:, 0:32].rearrange("p (m t) -> p m t", t=4)[:, :, 0:3],
                axis=X_AX, op=ADD)
            nc.vector.tensor_reduce(
                out=stat1a[:, 8:16].rearrange("p (m o) -> p m o", o=1),
                in_=stat1p[:, 32:64].rearrange("p (m t) -> p m t", t=4)[:, :, 0:3],
                axis=X_AX, op=ADD)
            stat1b = sb.tile([128, 16], F32)
            nc.vector.tensor_copy(
                out=stat1b[:, 0:8].rearrange("p (m o) -> p m o", o=1),
                in_=stat1p[:, 0:32].rearrange("p (m t) -> p m t", t=4)[:, :, 3:4])
            nc.vector.tensor_copy(
                out=stat1b[:, 8:16].rearrange("p (m o) -> p m o", o=1),
                in_=stat1p[:, 32:64].rearrange("p (m t) -> p m t", t=4)[:, :, 3:4])

            bi1a = dr.tile([128, 16], F32)
            bo1a = dr.tile([128, 16], F32)
            bi1b = dr.tile([128, 16], F32)
            bo1b = dr.tile([128, 16], F32)
            nc.sync.dma_start(out=bi1a[:], in_=stat1a[:])
            nc.sync.dma_start(out=bi1b[:], in_=stat1b[:])
            if sim_mode:
                nc.gpsimd.dma_start(out=bo1a[:], in_=bi1a[:])
                nc.gpsimd.dma_start(out=bo1b[:], in_=bi1b[:])
            else:
                nc.gpsimd.collective_compute(
                    "AllReduce", ADD, replica_groups=[list(range(NCORES))],
                    ins=[bi1a.opt()], outs=[bo1a.opt()])
                nc.gpsimd.collective_compute(
                    "AllReduce", ADD, replica_groups=[list(range(NCORES))],
                    ins=[bi1b.opt()], outs=[bo1b.opt()])
            ast1a = sb.tile([128, 16], F32)
            nc.sync.dma_start(out=ast1a[:], in_=bo1a[:])
            ast1b = sb.tile([128, 16], F32)
            nc.sync.dma_start(out=ast1b[:], in_=bo1b[:])
            nc.vector.tensor_tensor(out=stat1[:], in0=ast1a[:], in1=ast1b[:], op=ADD)

            # BN1 affine: s = g / sqrt(var+eps), t = be - mu*s
            mu1 = sb.tile([128, 8], F32)
            nc.vector.tensor_scalar(out=mu1[:], in0=stat1[:, 0:8], scalar1=1.0 / B,
                                    scalar2=None, op0=MUL)
            var1 = sb.tile([128, 8], F32)
            nc.vector.tensor_scalar(out=var1[:], in0=stat1[:, 8:16], scalar1=1.0 / B,
                                    scalar2=None, op0=MUL)
            musq1 = sb.tile([128, 8], F32)
            nc.vector.tensor_tensor(out=musq1[:], in0=mu1[:], in1=mu1[:], op=MUL)
            nc.vector.tensor_tensor(out=var1[:], in0=var1[:], in1=musq1[:], op=SUB)
            sd1 = sb.tile([128, 8], F32)
            nc.scalar.activation(out=sd1[:], in_=var1[:], func=AF.Sqrt, bias=eps_bn[:])
            inv1 = sb.tile([128, 8], F32)
            nc.vector.reciprocal(out=inv1[:], in_=sd1[:])
            s1 = sb.tile([128, 8], F32)
            nc.vector.tensor_tensor(out=s1[:], in0=g1[:], in1=inv1[:], op=MUL)
            t1 = sb.tile([128, 8], F32)
            nc.vector.tensor_tensor(out=t1[:], in0=mu1[:], in1=s1[:], op=MUL)
            nc.vector.tensor_tensor(out=t1[:], in0=be1[:], in1=t1[:], op=SUB)
            s1sq = sb.tile([128, 8], F32)
            nc.vector.tensor_tensor(out=s1sq[:], in0=s1[:], in1=s1[:], op=MUL)
            st1 = sb.tile([128, 8], F32)   # 2*s*t
            nc.vector.tensor_tensor(out=st1[:], in0=s1[:], in1=t1[:], op=MUL)
            nc.vector.tensor_scalar(out=st1[:], in0=st1[:], scalar1=2.0,
                                    scalar2=None, op0=MUL)
            tsq1 = sb.tile([128, 8], F32)
            nc.vector.tensor_tensor(out=tsq1[:], in0=t1[:], in1=t1[:], op=MUL)
            # sum(t^2) over all features -> [1,1], folded into the Sqrt bias
            tsq1p = ps.tile([1, 8], F32, tag="cs", bufs=1)
            nc.tensor.matmul(tsq1p[:], lhsT=ones_f32c[:], rhs=tsq1[:], start=True, stop=True)
            bias_d1 = sb.tile([1, 1], F32)
            nc.vector.tensor_reduce(
                out=bias_d1[:].rearrange("p (o c) -> p o c", o=1),
                in_=tsq1p[:].rearrange("p (o c) -> p o c", o=1), axis=X_AX, op=ADD)
            nc.vector.tensor_tensor(out=bias_d1[:], in0=bias_d1[:], in1=epsd1_row[:], op=ADD)

            y1 = sb.tile([128, 8 * D2], BF)
            for m in range(8):
                nc.vector.tensor_scalar(
                    out=y1[:, m * D2:(m + 1) * D2], in0=z1T[:, m * D2:(m + 1) * D2],
                    scalar1=s1[:, m:m + 1], scalar2=t1[:, m:m + 1], op0=MUL, op1=ADD)

            # ---------- Dice 1 (feature mean/var via PE ones-matmuls, bf16) ----------
            avgp = ps.tile([1, BL], F32, tag="cs", bufs=1)
            for m in range(8):
                nc.tensor.matmul(avgp[:], lhsT=ones_d1[:], rhs=y1[:, m * D2:(m + 1) * D2],
                                 start=(m == 0), stop=(m == 7))
            avgrow = sb.tile([1, BL], BF)
            nc.vector.tensor_copy(out=avgrow[:], in_=avgp[:])
            avgb = ps.tile([128, BL], F32, tag="bc")
            nc.tensor.matmul(avgb[:], lhsT=onesrow_bf[:], rhs=avgrow[:], start=True, stop=True)
            avgb_sb = sb.tile([128, BL], BF)
            nc.scalar.activation(out=avgb_sb[:], in_=avgb[:], func=AF.Copy)

            diff1 = sb.tile([128, 8 * D2], BF)
            for m in range(8):
                nc.vector.tensor_tensor(out=diff1[:, m * D2:(m + 1) * D2],
                                        in0=y1[:, m * D2:(m + 1) * D2],
                                        in1=avgb_sb[:], op=SUB)

            # var per sample = sum(y^2) - D1*avg^2 (+ eps*D1 + sum(t^2) in bias)
            # with sum(y^2) = sum(s^2 z^2) + sum(2st z) over the z drains
            s1sq_b = sb.tile([128, 8], BF)
            nc.vector.tensor_copy(out=s1sq_b[:], in_=s1sq[:])
            st1_b = sb.tile([128, 8], BF)
            nc.vector.tensor_copy(out=st1_b[:], in_=st1[:])
            varp = ps.tile([1, BL], F32, tag="cs", bufs=1)
            for m in range(8):
                nc.tensor.matmul(varp[:], lhsT=s1sq_b[:, m:m + 1],
                                 rhs=z1sq[:, m * D2:(m + 1) * D2],
                                 start=(m == 0), stop=False)
                nc.tensor.matmul(varp[:], lhsT=st1_b[:, m:m + 1],
                                 rhs=z1T[:, m * D2:(m + 1) * D2],
                                 start=False, stop=(m == 7))
            avgsq = sb.tile([1, BL], F32)
            nc.vector.tensor_tensor(out=avgsq[:], in0=avgrow[:], in1=avgrow[:], op=MUL)
            vrow = sb.tile([1, BL], F32)
            nc.vector.scalar_tensor_tensor(out=vrow[:], in0=avgsq[:], scalar=-float(D1),
                                           in1=varp[:], op0=MUL, op1=ADD)
            sqrow = sb.tile([1, BL], F32)
            nc.scalar.activation(out=sqrow[:], in_=vrow[:], func=AF.Sqrt, bias=bias_d1[:])
            rstd = sb.tile([1, BL], F32)
            nc.vector.reciprocal(out=rstd[:], in_=sqrow[:])
            rstdb16 = sb.tile([1, BL], BF)
            nc.vector.tensor_copy(out=rstdb16[:], in_=rstd[:])
            rstdb = ps.tile([128, BL], F32, tag="bc")
            nc.tensor.matmul(rstdb[:], lhsT=onesrow_bf[:], rhs=rstdb16[:], start=True, stop=True)
            rstdb_sb = sb.tile([128, BL], BF)
            nc.scalar.activation(out=rstdb_sb[:], in_=rstdb[:], func=AF.Copy)

            for m in range(8):
                sl = slice(m * D2, (m + 1) * D2)
                nc.vector.tensor_tensor(out=diff1[:, sl], in0=diff1[:, sl], in1=rstdb_sb[:], op=MUL)
                nc.scalar.activation(out=diff1[:, sl], in_=diff1[:, sl], func=AF.Sigmoid)
                nc.vector.tensor_scalar(out=diff1[:, sl], in0=diff1[:, sl],
                                        scalar1=oma1[:], scalar2=a1s[:], op0=MUL, op1=ADD)
                nc.vector.tensor_tensor(out=y1[:, sl], in0=y1[:, sl], in1=diff1[:, sl], op=MUL)

            # ---------- layer 2 ----------
            z2T = sb.tile([128, 4 * D2], BF)
            z2sq = sb.tile([128, 4 * D2], BF)
            stat2 = sb.tile([128, 8], F32)
            for m in range(4):
                zp = ps.tile([128, D2], F32, tag="zps", bufs=3)
                for k in range(8):
                    nc.tensor.matmul(zp[:], lhsT=w2[:, k * D2 + m * 128:k * D2 + (m + 1) * 128],
                                     rhs=y1[:, k * D2:(k + 1) * D2],
                                     start=(k == 0), stop=(k == 7))
                nc.scalar.activation(out=z2T[:, m * D2:(m + 1) * D2], in_=zp[:], func=AF.Copy,
                                     accum_out=stat2[:, m:m + 1])
                nc.scalar.activation(out=z2sq[:, m * D2:(m + 1) * D2], in_=zp[:], func=AF.Square,
                                     accum_out=stat2[:, 4 + m:5 + m])

            bi2 = dr.tile([128, 8], F32)
            bo2 = dr.tile([128, 8], F32)
            nc.sync.dma_start(out=bi2[:], in_=stat2[:])
            if sim_mode:
                nc.gpsimd.dma_start(out=bo2[:], in_=bi2[:])
            else:
                nc.gpsimd.collective_compute(
                    "AllReduce", ADD, replica_groups=[list(range(NCORES))],
                    ins=[bi2.opt()], outs=[bo2.opt()])
            ast2 = sb.tile([128, 8], F32)
            nc.sync.dma_start(out=ast2[:], in_=bo2[:])

            mu2 = sb.tile([128, 4], F32)
            nc.vector.tensor_scalar(out=mu2[:], in0=ast2[:, 0:4], scalar1=1.0 / B,
                                    scalar2=None, op0=MUL)
            var2 = sb.tile([128, 4], F32)
            nc.vector.tensor_scalar(out=var2[:], in0=ast2[:, 4:8], scalar1=1.0 / B,
                                    scalar2=None, op0=MUL)
            musq2 = sb.tile([128, 4], F32)
            nc.vector.tensor_tensor(out=musq2[:], in0=mu2[:], in1=mu2[:], op=MUL)
            nc.vector.tensor_tensor(out=var2[:], in0=var2[:], in1=musq2[:], op=SUB)
            sd2 = sb.tile([128, 4], F32)
            nc.scalar.activation(out=sd2[:], in_=var2[:], func=AF.Sqrt, bias=eps_bn[:])
            inv2 = sb.tile([128, 4], F32)
            nc.vector.reciprocal(out=inv2[:], in_=sd2[:])
            s2 = sb.tile([128, 4], F32)
            nc.vector.tensor_tensor(out=s2[:], in0=g2[:], in1=inv2[:], op=MUL)
            t2 = sb.tile([128, 4], F32)
            nc.vector.tensor_tensor(out=t2[:], in0=mu2[:], in1=s2[:], op=MUL)
            nc.vector.tensor_tensor(out=t2[:], in0=be2[:], in1=t2[:], op=SUB)
            s2sq = sb.tile([128, 4], F32)
            nc.vector.tensor_tensor(out=s2sq[:], in0=s2[:], in1=s2[:], op=MUL)
            st2 = sb.tile([128, 4], F32)
            nc.vector.tensor_tensor(out=st2[:], in0=s2[:], in1=t2[:], op=MUL)
            nc.vector.tensor_scalar(out=st2[:], in0=st2[:], scalar1=2.0,
                                    scalar2=None, op0=MUL)
            tsq2 = sb.tile([128, 4], F32)
            nc.vector.tensor_tensor(out=tsq2[:], in0=t2[:], in1=t2[:], op=MUL)
            tsq2p = ps.tile([1, 4], F32, tag="cs", bufs=1)
            nc.tensor.matmul(tsq2p[:], lhsT=ones_f32c[:], rhs=tsq2[:], start=True, stop=True)
            bias_d2 = sb.tile([1, 1], F32)
            nc.vector.tensor_reduce(
                out=bias_d2[:].rearrange("p (o c) -> p o c", o=1),
                in_=tsq2p[:].rearrange("p (o c) -> p o c", o=1), axis=X_AX, op=ADD)
            nc.vector.tensor_tensor(out=bias_d2[:], in0=bias_d2[:], in1=epsd2_row[:], op=ADD)
            s2sq_b = sb.tile([128, 4], BF)
            nc.vector.tensor_copy(out=s2sq_b[:], in_=s2sq[:])
            st2_b = sb.tile([128, 4], BF)
            nc.vector.tensor_copy(out=st2_b[:], in_=st2[:])

            y2 = sb.tile([128, 4 * D2], BF)   # keep raw z2T for the var matmuls
            for m in range(4):
                sl = slice(m * D2, (m + 1) * D2)
                nc.vector.tensor_scalar(
                    out=y2[:, sl], in0=z2T[:, sl],
                    scalar1=s2[:, m:m + 1], scalar2=t2[:, m:m + 1], op0=MUL, op1=ADD)

            # ---------- Dice 2 (bf16) ----------
            avgp2 = ps.tile([1, BL], F32, tag="cs", bufs=1)
            for m in range(4):
                nc.tensor.matmul(avgp2[:], lhsT=ones_d2[:], rhs=y2[:, m * D2:(m + 1) * D2],
                                 start=(m == 0), stop=(m == 3))
            avgrow2 = sb.tile([1, BL], BF)
            nc.vector.tensor_copy(out=avgrow2[:], in_=avgp2[:])
            avgb2 = ps.tile([128, BL], F32, tag="bc")
            nc.tensor.matmul(avgb2[:], lhsT=onesrow_bf[:], rhs=avgrow2[:], start=True, stop=True)
            avgb2_sb = sb.tile([128, BL], BF)
            nc.scalar.activation(out=avgb2_sb[:], in_=avgb2[:], func=AF.Copy)

            diff2 = sb.tile([128, 4 * D2], BF)
            for m in range(4):
                nc.vector.tensor_tensor(out=diff2[:, m * D2:(m + 1) * D2],
                                        in0=y2[:, m * D2:(m + 1) * D2], in1=avgb2_sb[:], op=SUB)
            varp2 = ps.tile([1, BL], F32, tag="cs", bufs=1)
            for m in range(4):
                nc.tensor.matmul(varp2[:], lhsT=s2sq_b[:, m:m + 1],
                                 rhs=z2sq[:, m * D2:(m + 1) * D2],
                                 start=(m == 0), stop=False)
                nc.tensor.matmul(varp2[:], lhsT=st2_b[:, m:m + 1],
                                 rhs=z2T[:, m * D2:(m + 1) * D2],
                                 start=False, stop=(m == 3))
            avgsq2 = sb.tile([1, BL], F32)
            nc.vector.tensor_tensor(out=avgsq2[:], in0=avgrow2[:], in1=avgrow2[:], op=MUL)
            vrow2 = sb.tile([1, BL], F32)
            nc.vector.scalar_tensor_tensor(out=vrow2[:], in0=avgsq2[:], scalar=-float(D2),
                                           in1=varp2[:], op0=MUL, op1=ADD)
            sqrow2 = sb.tile([1, BL], F32)
            nc.scalar.activation(out=sqrow2[:], in_=vrow2[:], func=AF.Sqrt, bias=bias_d2[:])
            rstd2 = sb.tile([1, BL], F32)
            nc.vector.reciprocal(out=rstd2[:], in_=sqrow2[:])
            rstd2b = sb.tile([1, BL], BF)
            nc.vector.tensor_copy(out=rstd2b[:], in_=rstd2[:])
            rstdb2 = ps.tile([128, BL], F32, tag="bc")
            nc.tensor.matmul(rstdb2[:], lhsT=onesrow_bf[:], rhs=rstd2b[:], start=True, stop=True)
            rstdb2_sb = sb.tile([128, BL], BF)
            nc.scalar.activation(out=rstdb2_sb[:], in_=rstdb2[:], func=AF.Copy)

            for m in range(4):
                sl = slice(m * D2, (m + 1) * D2)
                nc.vector.tensor_tensor(out=diff2[:, sl], in0=diff2[:, sl], in1=rstdb2_sb[:], op=MUL)
                nc.scalar.activation(out=diff2[:, sl], in_=diff2[:, sl], func=AF.Sigmoid)
                nc.vector.tensor_scalar(out=diff2[:, sl], in0=diff2[:, sl],
                                        scalar1=oma2[:], scalar2=a2s[:], op0=MUL, op1=ADD)
                nc.vector.tensor_tensor(out=y2[:, sl], in0=y2[:, sl], in1=diff2[:, sl], op=MUL)

            # ---------- layer 3: out row = W3^T y2 + b3 ----------
            z3p = ps.tile([1, BL], F32, tag="cs", bufs=1)
            for k in range(4):
                nc.tensor.matmul(z3p[:], lhsT=w3[:, k:k + 1], rhs=y2[:, k * D2:(k + 1) * D2],
                                 start=(k == 0), stop=(k == 3))
            z3row = sb.tile([1, BL], F32)
            nc.vector.tensor_scalar(out=z3row[:], in0=z3p[:], scalar1=b3s[0:1, 0:1],
                                    scalar2=None, op0=ADD)
            nc.sync.dma_start(out=outd[:], in_=z3row[:])

    nc.compile()
    return nc


def _get_prog():
    global _PROG
    if _PROG is None:
        _PROG = _build()
    return _PROG


def kernel(items, users, history_users, item_table, user_table,
           W1, b1, g1, be1, a1, W2, b2, g2, be2, a2, W3, b3):
    nc = _get_prog()

    items = np.asarray(items).astype(np.int32)
    users = np.asarray(users).astype(np.int32)
    hist = np.asarray(history_users).astype(np.int32)
    item_table = np.ascontiguousarray(
        np.asarray(item_table, dtype=np.float32)).astype(
            ml_dtypes.bfloat16).reshape(N_ITEM * D // 4096, 4096)
    user_table = np.ascontiguousarray(
        np.asarray(user_table, dtype=np.float32)).astype(
            ml_dtypes.bfloat16).reshape(M_USER * D // 4096, 4096)
    W1 = np.asarray(W1, dtype=np.float32)
    W2 = np.asarray(W2, dtype=np.float32)
    W3 = np.asarray(W3, dtype=np.float32)

    # host-side weight reshapes (shared across cores)
    w1sb = W1.reshape(3, 128, D1).transpose(1, 0, 2).reshape(128, 3 * D1)
    w1sb = np.ascontiguousarray(w1sb).astype(ml_dtypes.bfloat16)
    w2sb = W2.reshape(8, 128, D2).transpose(1, 0, 2).reshape(128, 8 * D2)
    w2sb = np.ascontiguousarray(w2sb).astype(ml_dtypes.bfloat16)
    w3sb = np.ascontiguousarray(W3.reshape(4, 128).T).astype(ml_dtypes.bfloat16)
    g1r = np.ascontiguousarray(np.asarray(g1, np.float32).reshape(8, 128).T)
    be1r = np.ascontiguousarray(np.asarray(be1, np.float32).reshape(8, 128).T)
    g2r = np.ascontiguousarray(np.asarray(g2, np.float32).reshape(4, 128).T)
    be2r = np.ascontiguousarray(np.asarray(be2, np.float32).reshape(4, 128).T)
    a1c = np.full((128, 1), np.float32(np.asarray(a1).ravel()[0]), np.float32)
    a2c = np.full((128, 1), np.float32(np.asarray(a2).ravel()[0]), np.float32)
    b3c = np.full((1, 1), np.float32(np.asarray(b3).ravel()[0]), np.float32)

    in_maps = []
    for c in range(NCORES):
        sl = slice(c * BL, (c + 1) * BL)
        idx_hist = (hist[sl] * D).reshape(T, 128, H).transpose(1, 0, 2).reshape(128, T * H)
        idx_u = (users[sl] * D).reshape(T, 128).T
        idx_i = (items[sl] * D).reshape(T, 128).T
        in_maps.append({
            "user_table": user_table, "item_table": item_table,
            "idx_hist": np.ascontiguousarray(idx_hist),
            "idx_u": np.ascontiguousarray(idx_u),
            "idx_i": np.ascontiguousarray(idx_i),
            "w1sb": w1sb, "w2sb": w2sb, "w3sb": w3sb,
            "g1r": g1r, "be1r": be1r, "g2r": g2r, "be2r": be2r,
            "a1c": a1c, "a2c": a2c, "b3c": b3c,
        })

    res = run_bass_kernel_spmd(nc, in_maps, core_ids=list(range(NCORES)))
    out = np.concatenate(
        [np.asarray(res.results[c]["out"], np.float32).reshape(BL, 1) for c in range(NCORES)],
        axis=0)
    return out
